# revision 1
# baseline (speedup 1.0000x reference)
"""Trainium2 Bass kernel for nn_BestNet_46196668236142 (LRU block).

Pipeline per token: LN1 -> leaky -> complex diagonal recurrence over T
-> y = Re(C h) + D z -> leaky(LN2) -> MLP -> LN3 -> +skip.

Strategy:
- Data-parallel: shard B=32 across 8 cores (4 samples/core).
- The complex recurrence h_t = lam*h_{t-1} + u_t (lam = r*e^{i th}) is
  decoupled into two REAL per-channel scans via polar rotation:
      g_t = e^{-i th t} h_t   =>   g_t = r * g_{t-1} + e^{-i th t} u_t
  which maps onto the HW tensor_tensor_scan (op0=mult, op1=add) along
  the free (time) axis, n on partitions. Pre/post rotations use
  host-precomputed cos/sin tables; the post-rotation recombines into
  hr = Re(h) and -Im(h) so the C projection needs only 2 streams
  (plus D and MLP: 6 fp32 matmul streams total, the minimum).
- Both n-halves (d_state=256 = 2x128 partitions) are packed side by
  side in the free dim ([128, 2*CT] tiles) so every rotation runs as
  one wide DVE op; scans slice per half (a scan may not cross halves).
- Chunked over time (CT=512) with a tiny [P,1] carry between chunks,
  computed on the scalar engine via per-partition scale/bias.
- Software pipeline over macro-steps i = (chunk, sample): each step
  emits s1(i+1) | s5ln(i-2) | s2(i) | s5tp+s6mm(i-2) | s34a(i) |
  s6ln(i-2) | s5mm(i-1) | s34b(i). This order keeps every
  cross-engine consumer reading data >= 1 step old (the only
  same-step edge, Bproj->prerot, is time-aligned), so the in-order
  per-engine queues never head-of-line block on a fresh producer.
- Engine placement: PE matmuls/transposes (fp32 LOW_HIGH, 2 passes -
  required: the rel-err budget rules out bf16/fp32r); DVE rotations +
  scans + LN stats + reciprocal; Act engine does sqrt, LN applies,
  fused leaky via Prelu (parametric_relu shares the sqrt activation
  table -> zero ACT_TABLE_LOADs), PSUM drains, and the carry math.
- Matmul results are staged PSUM->SBUF by quick scalar copies so the
  2-slot PSUM accumulator ring recycles in ~0.6us instead of blocking
  on the LN chain; transposes pair into [128,256] PSUM tiles drained
  by one strided copy.
"""

import os
import sys

import numpy as np

for _p in ("/opt/trn_rl_repo", "/root/.axon_site/_ro/trn_rl_repo"):
    if os.path.isdir(_p) and _p not in sys.path:
        sys.path.insert(0, _p)

import concourse.bass as bass
import concourse.mybir as mybir
from concourse import bacc, masks, tile
from concourse.bass_utils import run_bass_kernel_spmd

B, T, D, N = 32, 4096, 256, 256
NCORES = 8
BS = B // NCORES            # batches per core
CT = 512                    # time chunk
NSUB = CT // 128            # token subtiles per chunk
NCH = T // CT               # chunks per batch
EPS = 1e-5
SLOPE = 0.01
F32 = mybir.dt.float32
AO = mybir.AluOpType
AF = mybir.ActivationFunctionType

_PROG_CACHE = {}


def _build_program(flags):
    """flags = (g1, g2, g3, bias, mask) booleans for the general path."""
    g1, g2, g3, use_bias, use_mask = flags
    nc = bacc.Bacc(None, target_bir_lowering=False)

    x_d = nc.declare_dram_parameter("x", [BS, T, D], F32, isOutput=False)
    q0r_d = nc.declare_dram_parameter("q0r", [BS, N], F32, isOutput=False)
    q0i_d = nc.declare_dram_parameter("q0i", [BS, N], F32, isOutput=False)
    cos_d = nc.declare_dram_parameter("cosj", [N, CT], F32, isOutput=False)
    sin_d = nc.declare_dram_parameter("sinj", [N, CT], F32, isOutput=False)
    cneg_d = nc.declare_dram_parameter("cneg", [N, CT], F32, isOutput=False)
    sneg_d = nc.declare_dram_parameter("sneg", [N, CT], F32, isOutput=False)
    rbc_d = nc.declare_dram_parameter("rbc", [N, CT], F32, isOutput=False)
    ecl_d = nc.declare_dram_parameter("ecl", [N, 1], F32, isOutput=False)
    esl_d = nc.declare_dram_parameter("esl", [N, 1], F32, isOutput=False)
    nesl_d = nc.declare_dram_parameter("nesl", [N, 1], F32, isOutput=False)
    brt_d = nc.declare_dram_parameter("BrT", [D, N], F32, isOutput=False)
    bit_d = nc.declare_dram_parameter("BiT", [D, N], F32, isOutput=False)
    crt_d = nc.declare_dram_parameter("CrT", [N, D], F32, isOutput=False)
    cit_d = nc.declare_dram_parameter("CiT", [N, D], F32, isOutput=False)
    dt_d = nc.declare_dram_parameter("DT", [D, N], F32, isOutput=False)
    mt_d = nc.declare_dram_parameter("MT", [N, D], F32, isOutput=False)
    out_d = nc.declare_dram_parameter("out", [BS, T, D], F32, isOutput=True)

    if use_mask:
        d0_d = nc.declare_dram_parameter("d0tab", [BS, N, T], F32, isOutput=False)
    gb_params = {}
    for name, on in (("g1", g1), ("b1", g1), ("g2", g2), ("b2", g2),
                     ("g3", g3), ("b3", g3), ("mb", use_bias)):
        if on:
            gb_params[name] = nc.declare_dram_parameter(name + "bc", [128, D], F32)

    from contextlib import ExitStack

    with tile.TileContext(nc) as tc, ExitStack() as ctx:
        cpool = ctx.enter_context(tc.tile_pool(name="consts", bufs=1))

        _cn = [0]

        def cload(dram, shape):
            _cn[0] += 1
            t = cpool.tile(shape, F32, name=f"const{_cn[0]}",
                           tag=f"const{_cn[0]}")
            nc.sync.dma_start(t[:], dram)
            return t

        epst = cpool.tile([128, 1], F32)
        nc.gpsimd.memset(epst[:], EPS)
        cos2 = sin2 = cng2 = sng2 = rbc2 = None
        ecl = esl = nesl = brt = bit = crt = cit = dts = mts = gbt = None
        ident = None

        def load_consts():
            nonlocal cos2, sin2, cng2, sng2, rbc2, ecl, esl, nesl
            nonlocal brt, bit, crt, cit, dts, mts, gbt, ident
            # constants: tables with both n-halves side by side in the free dim
            def cload2(dram):
                _cn[0] += 1
                t = cpool.tile([128, 2 * CT], F32, name=f"const{_cn[0]}",
                               tag=f"const{_cn[0]}")
                for p in range(2):
                    nc.sync.dma_start(t[:, p * CT:(p + 1) * CT],
                                      dram[p * 128:(p + 1) * 128, :])
                return t

            cos2 = cload2(cos_d)
            sin2 = cload2(sin_d)
            cng2 = cload2(cneg_d)
            sng2 = cload2(sneg_d)
            rbc2 = cload2(rbc_d)
            ecl = [cload(ecl_d[p * 128:(p + 1) * 128, :], [128, 1]) for p in range(2)]
            esl = [cload(esl_d[p * 128:(p + 1) * 128, :], [128, 1]) for p in range(2)]
            nesl = [cload(nesl_d[p * 128:(p + 1) * 128, :], [128, 1]) for p in range(2)]
            brt = [cload(brt_d[k * 128:(k + 1) * 128, :], [128, N]) for k in range(2)]
            bit = [cload(bit_d[k * 128:(k + 1) * 128, :], [128, N]) for k in range(2)]
            crt = [cload(crt_d[p * 128:(p + 1) * 128, :], [128, D]) for p in range(2)]
            cit = [cload(cit_d[p * 128:(p + 1) * 128, :], [128, D]) for p in range(2)]
            dts = [cload(dt_d[k * 128:(k + 1) * 128, :], [128, N]) for k in range(2)]
            mts = [cload(mt_d[p * 128:(p + 1) * 128, :], [128, D]) for p in range(2)]
            gbt = {k: cload(v[:, :], [128, D]) for k, v in gb_params.items()}
            ident = cpool.tile([128, 128], F32)
            masks.make_identity(nc, ident[:])


        xin = ctx.enter_context(tc.tile_pool(name="xin", bufs=3))
        statp = ctx.enter_context(tc.tile_pool(name="stat", bufs=48))
        zskip = ctx.enter_context(tc.tile_pool(name="zskip", bufs=18))
        zlp = ctx.enter_context(tc.tile_pool(name="zl", bufs=8))
        ztp = ctx.enter_context(tc.tile_pool(name="zt", bufs=4))
        ptr = ctx.enter_context(
            tc.tile_pool(name="ptr", bufs=2, space=bass.MemorySpace.PSUM))
        pu = ctx.enter_context(
            tc.tile_pool(name="pu", bufs=2, space=bass.MemorySpace.PSUM))
        tmpv = ctx.enter_context(tc.tile_pool(name="tmpv", bufs=4))
        tmpg = ctx.enter_context(tc.tile_pool(name="tmpg", bufs=4))
        wp = ctx.enter_context(tc.tile_pool(name="w", bufs=2))
        gp = ctx.enter_context(tc.tile_pool(name="g", bufs=3))
        gip = ctx.enter_context(tc.tile_pool(name="gi", bufs=40))
        ap_ = ctx.enter_context(tc.tile_pool(name="astr", bufs=6))
        pacc = ctx.enter_context(
            tc.tile_pool(name="pacc", bufs=2, space=bass.MemorySpace.PSUM))
        psp = ctx.enter_context(tc.tile_pool(name="ps", bufs=4))
        yl2p = ctx.enter_context(tc.tile_pool(name="yl2", bufs=4))
        y2tp = ctx.enter_context(tc.tile_pool(name="y2t", bufs=2))
        yop = ctx.enter_context(tc.tile_pool(name="yo", bufs=4))
        if use_mask:
            d0p = ctx.enter_context(tc.tile_pool(name="d0p", bufs=3))

        def ln4(src_all):
            """Batched LN stats for a [128, 4*D] tile holding 4 subtile
            inputs: ONE grouped bn_stats, then aggr/sqrt/recip batched so
            the in-order DVE queue never stalls on a sqrt round-trip."""
            st24 = statp.tile([128, 4 * 6], F32, name="st24", tag="st24")
            for s in range(4):
                nc.vector.bn_stats(
                    st24[:, s * 6:(s + 1) * 6],
                    src_all[:, s * D:(s + 1) * D])
            mv8 = statp.tile([128, 8], F32, name="mv8", tag="mv8")
            for s in range(4):
                nc.vector.bn_aggr(mv8[:, 2 * s:2 * s + 2],
                                  st24[:, s * 6:(s + 1) * 6])
            std4 = statp.tile([128, 4], F32, name="std4", tag="std4")
            for s in range(4):
                nc.scalar.activation(std4[:, s:s + 1], mv8[:, 2 * s + 1:2 * s + 2],
                                     AF.Sqrt, bias=epst[:])
            rstd4 = statp.tile([128, 4], F32, name="std4", tag="std4")
            nc.vector.reciprocal(rstd4[:], std4[:])
            # negate all 4 means in one scalar op, then one [128,4] multiply
            nmu4 = statp.tile([128, 4], F32, name="std4", tag="std4")
            nc.scalar.activation(
                nmu4[:].rearrange("p (s x) -> p s x", x=1),
                mv8[:].rearrange("p (s x) -> p s x", x=2)[:, :, 0:1],
                AF.Identity, scale=-1.0)
            nmr4 = statp.tile([128, 4], F32, name="std4", tag="std4")
            nc.vector.tensor_mul(nmr4[:], nmu4[:], rstd4[:])
            return [(rstd4[:, s:s + 1], nmr4[:, s:s + 1]) for s in range(4)]

        def ln_scale_bias(src_ap):
            """Return (rstd, negmu_rstd) [128,1] tiles for a [128, D] input."""
            st6 = statp.tile([128, 6], F32)
            nc.vector.bn_stats(st6[:], src_ap)
            mv = statp.tile([128, 2], F32)
            nc.vector.bn_aggr(mv[:], st6[:])
            std = statp.tile([128, 1], F32, name="cst", tag="cst")
            nc.scalar.activation(std[:], mv[:, 1:2], AF.Sqrt, bias=epst[:])
            rstd = statp.tile([128, 1], F32, name="cst", tag="cst")
            nc.vector.reciprocal(rstd[:], std[:])
            nmr = statp.tile([128, 1], F32)
            nc.vector.scalar_tensor_tensor(
                nmr[:], mv[:, 0:1], -1.0, rstd[:], op0=AO.mult, op1=AO.mult)
            return rstd, nmr

        # per-batch persistent state
        ginit = {}

        def load_ginit():
            for b in range(BS):
                for p in range(2):
                    for comp, src_d in ((0, q0r_d), (1, q0i_d)):
                        t = gip.tile([128, 1], F32, name="giq", tag="giq")
                        nc.sync.dma_start(
                            t[:], src_d[b, p * 128:(p + 1) * 128])
                        ginit[(b, p, comp)] = t
        zls = {}
        zts = {}
        zsks = {}
        pus = {}
        hrs = {}

        def emit_s1(i):
            """Load x, LN1 stats+apply, leaky -> zl, zsk."""
            c, b = divmod(i, BS)
            t0 = c * CT
            zl = []
            zsk = []
            xt4 = xin.tile([128, NSUB * D], F32, name="xt", tag="xt")
            for s in range(NSUB):
                nc.sync.dma_start(
                    xt4[:, s * D:(s + 1) * D],
                    x_d[b, t0 + s * 128:t0 + (s + 1) * 128, :])
            sb = ln4(xt4[:])
            for s in range(NSUB):
                xt = xt4[:, s * D:(s + 1) * D]
                rstd, nmr = sb[s]
                z = zskip.tile([128, D], F32)
                nc.scalar.activation(
                    z[:], xt, AF.Identity, bias=nmr, scale=rstd)
                if g1:
                    nc.vector.tensor_mul(z[:], z[:], gbt["g1"][:])
                    nc.vector.tensor_add(z[:], z[:], gbt["b1"][:])
                zsk.append(z)
                zlt = zlp.tile([128, D], F32, name="zl", tag="zl")
                if g1:
                    nc.vector.scalar_tensor_tensor(
                        zlt[:], z[:], SLOPE, z[:], op0=AO.mult, op1=AO.max)
                else:
                    nc.scalar.activation(
                        zlt[:], xt, AF.Prelu, bias=nmr, scale=rstd,
                        alpha=SLOPE)
                zl.append(zlt)
            zls[i] = zl
            zsks[i] = zsk

        def emit_s2(i):
            """Transpose zl -> zt; B projection -> u (PSUM)."""
            zl = zls.pop(i)
            # zt_all[:, k*CT + t] holds z^T for d-half k: [128, 2*CT]
            zt_all = ztp.tile([128, 2 * CT], F32, name="zt", tag="zt")
            zt = [zt_all[:, k * CT:(k + 1) * CT] for k in range(2)]
            ztv = zt_all[:].rearrange("p (k t) -> p k t", k=2)
            for s in range(NSUB):
                pt = ptr.tile([128, 256], F32, name="pt", tag="pt")
                for k in range(2):
                    nc.tensor.transpose(
                        pt[:, k * 128:(k + 1) * 128],
                        zl[s][:, k * 128:(k + 1) * 128], ident[:])
                nc.scalar.copy(
                    ztv[:, :, s * 128:(s + 1) * 128],
                    pt[:].rearrange("p (k t) -> p k t", k=2))
            u = {}
            for comp, bt in ((0, brt), (1, bit)):
                u2 = pu.tile([128, 2 * CT], F32, name="ut", tag="ut")
                for p in range(2):
                    for k in range(2):
                        nc.tensor.matmul(
                            u2[:, p * CT:(p + 1) * CT],
                            bt[k][:, p * 128:(p + 1) * 128], zt[k],
                            start=(k == 0), stop=(k == 1))
                u[comp] = u2
            zts[i] = zt
            pus[i] = u

        def emit_s34(i):
            """Pre-rotation (reads u from PSUM), scans, carry, post-rotation."""
            c, b = divmod(i, BS)
            t0 = c * CT
            u = pus.pop(i)
            hr = {}
            for p in range(2):
                if use_mask:
                    d0 = d0p.tile([128, CT], F32)
                    nc.sync.dma_start(
                        d0[:], d0_d[b, p * 128:(p + 1) * 128, t0:t0 + CT])
                    d0ap = d0[:]
                else:
                    d0ap = rbc[p][:]
                m1 = tmpv.tile([128, CT], F32, name="tv", tag="tv")
                nc.vector.tensor_mul(m1[:], cosj[p][:], u[(p, 0)][:])
                m2 = tmpv.tile([128, CT], F32, name="tv", tag="tv")
                nc.vector.tensor_mul(m2[:], sinj[p][:], u[(p, 1)][:])
                wr = wp.tile([128, CT], F32, name="w", tag="w")
                nc.vector.tensor_add(wr[:], m1[:], m2[:])
                m3 = tmpv.tile([128, CT], F32, name="tv", tag="tv")
                nc.vector.tensor_mul(m3[:], cosj[p][:], u[(p, 1)][:])
                m4 = tmpv.tile([128, CT], F32, name="tv", tag="tv")
                nc.vector.tensor_mul(m4[:], sneg[p][:], u[(p, 0)][:])
                wi = wp.tile([128, CT], F32, name="w", tag="w")
                nc.vector.tensor_add(wi[:], m3[:], m4[:])
                gr = gp.tile([128, CT], F32, name="g", tag="g")
                nc.vector.tensor_tensor_scan(
                    gr[:], d0ap, wr[:], ginit[(b, p, 0)][:],
                    op0=AO.mult, op1=AO.add)
                gi_t = gp.tile([128, CT], F32, name="g", tag="g")
                nc.vector.tensor_tensor_scan(
                    gi_t[:], d0ap, wi[:], ginit[(b, p, 1)][:],
                    op0=AO.mult, op1=AO.add)
                if c + 1 < NCH:
                    # carry to next chunk: ginit' = e^{i th L} * g_last,
                    # on the scalar engine via per-partition scale/bias:
                    #   ngr = grl*ecl + gil*(-esl); ngi = gil*ecl + grl*esl
                    grl = gr[:, CT - 1:CT]
                    gil = gi_t[:, CT - 1:CT]
                    tb = statp.tile([128, 1], F32, name="cst", tag="cst")
                    nc.scalar.activation(
                        tb[:], gil, AF.Identity, scale=nesl[p][:])
                    ngr = gip.tile([128, 1], F32, name="giq", tag="giq")
                    nc.scalar.activation(
                        ngr[:], grl, AF.Identity, scale=ecl[p][:],
                        bias=tb[:])
                    td = statp.tile([128, 1], F32, name="cst", tag="cst")
                    nc.scalar.activation(
                        td[:], grl, AF.Identity, scale=esl[p][:])
                    ngi = gip.tile([128, 1], F32, name="giq", tag="giq")
                    nc.scalar.activation(
                        ngi[:], gil, AF.Identity, scale=ecl[p][:],
                        bias=td[:])
                    ginit[(b, p, 0)] = ngr
                    ginit[(b, p, 1)] = ngi
                # post-rotation: hr = Re(h) = cos*gr - sin*gi
                #                hn = -Im(h) = -(sin*gr + cos*gi)
                q1 = tmpg.tile([128, CT], F32, name="tg", tag="tg")
                nc.vector.tensor_mul(q1[:], cosj[p][:], gr[:])
                q2 = tmpg.tile([128, CT], F32, name="tg", tag="tg")
                nc.vector.tensor_mul(q2[:], sneg[p][:], gi_t[:])
                hrt = ap_.tile([128, CT], F32, name="h", tag="h")
                nc.vector.tensor_add(hrt[:], q1[:], q2[:])
                q3 = tmpg.tile([128, CT], F32, name="tg", tag="tg")
                nc.vector.tensor_mul(q3[:], sneg[p][:], gr[:])
                q4 = tmpg.tile([128, CT], F32, name="tg", tag="tg")
                nc.vector.tensor_mul(q4[:], cneg[p][:], gi_t[:])
                hnt = ap_.tile([128, CT], F32, name="h", tag="h")
                nc.vector.tensor_add(hnt[:], q3[:], q4[:])
                hr[(p, 0)] = hrt
                hr[(p, 1)] = hnt
            hrs[i] = hr

        pss = {}
        yl2s = {}
        y2ts = {}
        p3ss = {}

        def emit_s5mm(i):
            """C/D projection matmuls -> pacc -> stage to SBUF (ps)."""
            zt = zts[i]
            hr2, hn2 = hrs.pop(i)
            ps4 = psp.tile([128, NSUB * D], F32, name="ps", tag="ps")
            for s in range(NSUB):
                sl = slice(s * 128, (s + 1) * 128)
                pt = pacc.tile([128, D], F32, name="pacc", tag="pacc")
                mms = []
                for p in range(2):
                    mms.append((hr2[:, p * CT + s * 128:p * CT + (s + 1) * 128],
                                crt[p][:]))
                for p in range(2):
                    mms.append((hn2[:, p * CT + s * 128:p * CT + (s + 1) * 128],
                                cit[p][:]))
                for k in range(2):
                    mms.append((zt[k][:, sl], dts[k][:]))
                for j, (lhs, rhs) in enumerate(mms):
                    nc.tensor.matmul(pt[:], lhs, rhs, start=(j == 0),
                                     stop=(j == len(mms) - 1))
                nc.scalar.copy(ps4[:, s * D:(s + 1) * D], pt[:])
            pss[i] = ps4

        def emit_s5ln(i):
            """LN2 + leaky off the SBUF-staged C/D results."""
            ps4 = pss.pop(i)
            yl4 = []
            sb = ln4(ps4[:])
            for s in range(NSUB):
                ps = ps4[:, s * D:(s + 1) * D]
                rstd, nmr = sb[s]
                yl2 = yl2p.tile([128, D], F32)
                if g2:
                    nc.scalar.activation(
                        yl2[:], ps, AF.Identity, bias=nmr, scale=rstd)
                    nc.vector.tensor_mul(yl2[:], yl2[:], gbt["g2"][:])
                    nc.vector.tensor_add(yl2[:], yl2[:], gbt["b2"][:])
                    nc.vector.scalar_tensor_tensor(
                        yl2[:], yl2[:], SLOPE, yl2[:], op0=AO.mult, op1=AO.max)
                else:
                    nc.scalar.activation(
                        yl2[:], ps, AF.Prelu, bias=nmr, scale=rstd,
                        alpha=SLOPE)
                yl4.append(yl2)
            yl2s[i] = yl4

        def emit_s5tp(i):
            """Transpose yl2 -> y2t."""
            yl4 = yl2s.pop(i)
            y2_all = y2tp.tile([128, 2 * CT], F32, name="y2t", tag="y2t")
            y2t = [y2_all[:, p * CT:(p + 1) * CT] for p in range(2)]
            y2v = y2_all[:].rearrange("p (k t) -> p k t", k=2)
            for s in range(NSUB):
                ptt = ptr.tile([128, 256], F32, name="pt", tag="pt")
                for p in range(2):
                    nc.tensor.transpose(
                        ptt[:, p * 128:(p + 1) * 128],
                        yl4[s][:, p * 128:(p + 1) * 128], ident[:])
                nc.scalar.copy(
                    y2v[:, :, s * 128:(s + 1) * 128],
                    ptt[:].rearrange("p (k t) -> p k t", k=2))
            y2ts[i] = y2t

        def emit_s6mm(i):
            """MLP matmuls -> pacc -> stage to SBUF."""
            y2t = y2ts.pop(i)
            p34 = psp.tile([128, NSUB * D], F32, name="ps", tag="ps")
            for s in range(NSUB):
                sl = slice(s * 128, (s + 1) * 128)
                p3 = pacc.tile([128, D], F32, name="pacc", tag="pacc")
                for p in range(2):
                    nc.tensor.matmul(p3[:], y2t[p][:, sl], mts[p][:],
                                     start=(p == 0), stop=(p == 1))
                nc.scalar.copy(p34[:, s * D:(s + 1) * D], p3[:])
            p3ss[i] = p34

        def emit_s6ln(i):
            """LN3 + skip + store."""
            c, b = divmod(i, BS)
            t0 = c * CT
            zsk = zsks.pop(i)
            p34 = p3ss.pop(i)
            del zts[i]
            if use_bias:
                for s in range(NSUB):
                    nc.vector.tensor_add(
                        p34[:, s * D:(s + 1) * D],
                        p34[:, s * D:(s + 1) * D], gbt["mb"][:])
            sb = ln4(p34[:])
            for s in range(NSUB):
                p3s = p34[:, s * D:(s + 1) * D]
                rstd, nmr = sb[s]
                yo = yop.tile([128, D], F32)
                nc.scalar.activation(
                    yo[:], p3s, AF.Identity, bias=nmr, scale=rstd)
                if g3:
                    nc.vector.tensor_mul(yo[:], yo[:], gbt["g3"][:])
                    nc.vector.tensor_add(yo[:], yo[:], gbt["b3"][:])
                nc.vector.tensor_add(yo[:], yo[:], zsk[s][:])
                nc.sync.dma_start(
                    out_d[b, t0 + s * 128:t0 + (s + 1) * 128, :], yo[:])

        def emit_s34a(i):
            """Pre-rotation (reads u from PSUM), scans, carry."""
            c, b = divmod(i, BS)
            t0 = c * CT
            u = pus.pop(i)
            if use_mask:
                d02 = d0p.tile([128, 2 * CT], F32, name="d0", tag="d0")
                for p in range(2):
                    nc.sync.dma_start(
                        d02[:, p * CT:(p + 1) * CT],
                        d0_d[b, p * 128:(p + 1) * 128, t0:t0 + CT])
                d0ap = d02[:]
            else:
                d0ap = rbc2[:]
            m1 = tmpv.tile([128, 2 * CT], F32, name="tv", tag="tv")
            nc.vector.tensor_mul(m1[:], cos2[:], u[0][:])
            m2 = tmpv.tile([128, 2 * CT], F32, name="tv", tag="tv")
            nc.vector.tensor_mul(m2[:], sin2[:], u[1][:])
            wr = wp.tile([128, 2 * CT], F32, name="w", tag="w")
            nc.vector.tensor_add(wr[:], m1[:], m2[:])
            m3 = tmpv.tile([128, 2 * CT], F32, name="tv", tag="tv")
            nc.vector.tensor_mul(m3[:], cos2[:], u[1][:])
            m4 = tmpv.tile([128, 2 * CT], F32, name="tv", tag="tv")
            nc.vector.tensor_mul(m4[:], sng2[:], u[0][:])
            wi = wp.tile([128, 2 * CT], F32, name="w", tag="w")
            nc.vector.tensor_add(wi[:], m3[:], m4[:])
            gr2 = gp.tile([128, 2 * CT], F32, name="g", tag="g")
            gi2 = gp.tile([128, 2 * CT], F32, name="g", tag="g")
            for p in range(2):
                cs = slice(p * CT, (p + 1) * CT)
                nc.vector.tensor_tensor_scan(
                    gr2[:, cs], d0ap[:, cs], wr[:, cs], ginit[(b, p, 0)][:],
                    op0=AO.mult, op1=AO.add)
                nc.vector.tensor_tensor_scan(
                    gi2[:, cs], d0ap[:, cs], wi[:, cs], ginit[(b, p, 1)][:],
                    op0=AO.mult, op1=AO.add)
                if c + 1 < NCH:
                    # carry: ginit' = e^{i th L} * g_last on the scalar
                    # engine via per-partition scale/bias:
                    #   ngr = grl*ecl + gil*(-esl); ngi = gil*ecl + grl*esl
                    e = (p + 1) * CT
                    grl = gr2[:, e - 1:e]
                    gil = gi2[:, e - 1:e]
                    tb = statp.tile([128, 1], F32, name="cst", tag="cst")
                    nc.scalar.activation(
                        tb[:], gil, AF.Identity, scale=nesl[p][:])
                    ngr = gip.tile([128, 1], F32, name="giq", tag="giq")
                    nc.scalar.activation(
                        ngr[:], grl, AF.Identity, scale=ecl[p][:],
                        bias=tb[:])
                    td = statp.tile([128, 1], F32, name="cst", tag="cst")
                    nc.scalar.activation(
                        td[:], grl, AF.Identity, scale=esl[p][:])
                    ngi = gip.tile([128, 1], F32, name="giq", tag="giq")
                    nc.scalar.activation(
                        ngi[:], gil, AF.Identity, scale=ecl[p][:],
                        bias=td[:])
                    ginit[(b, p, 0)] = ngr
                    ginit[(b, p, 1)] = ngi
            return (gr2, gi2)

        def emit_s34b(i, gg):
            """Post-rotation pairing: hr = Re(h), hn = -Im(h)."""
            gr2, gi2 = gg
            q1 = tmpg.tile([128, 2 * CT], F32, name="tg", tag="tg")
            nc.vector.tensor_mul(q1[:], cos2[:], gr2[:])
            q2 = tmpg.tile([128, 2 * CT], F32, name="tg", tag="tg")
            nc.vector.tensor_mul(q2[:], sng2[:], gi2[:])
            hr2 = ap_.tile([128, 2 * CT], F32, name="h", tag="h")
            nc.vector.tensor_add(hr2[:], q1[:], q2[:])
            q3 = tmpg.tile([128, 2 * CT], F32, name="tg", tag="tg")
            nc.vector.tensor_mul(q3[:], sng2[:], gr2[:])
            q4 = tmpg.tile([128, 2 * CT], F32, name="tg", tag="tg")
            nc.vector.tensor_mul(q4[:], cng2[:], gi2[:])
            hn2 = ap_.tile([128, 2 * CT], F32, name="h", tag="h")
            nc.vector.tensor_add(hn2[:], q3[:], q4[:])
            hrs[i] = (hr2, hn2)

        # Software pipeline. Per-step emission order is engineered so that
        # (a) each engine's in-order queue never buries a producer another
        # engine needs soon, and (b) every cross-engine edge except
        # Bproj(i)->prerot(i) reads data at least one macro-step old:
        #   s1(i+1)   V:LN1        S:apply/Prelu
        #   s5ln(i-2) V:LN2stats   S:Prelu        (off SBUF ps, 1 step old)
        #   s2(i)     PE:tp+Bproj  S:zcopies
        #   s5tp(i-2) PE:ytranspose S:y2tcopies
        #   s6mm(i-2) PE:MLP       S:p3s copies
        #   s34a(i)   V:prerot+scans+carry  (aligned with Bproj(i) finish)
        #   s6ln(i-2) V:LN3stats+skip S:apply  (MLP done by now)
        #   s5mm(i-1) PE:C/D       S:ps copies  (scan output 1 step old)
        #   s34b(i)   V:postrot
        NT = NCH * BS
        gg_cur = None
        for i in range(-1, NT + 2):
            if 0 <= i - 2 < NT:
                emit_s5ln(i - 2)
            if 0 <= i + 1 < NT:
                emit_s1(i + 1)
            if i == -1:
                load_consts()
                load_ginit()
            if 0 <= i < NT:
                emit_s2(i)
            if 0 <= i - 2 < NT:
                emit_s5tp(i - 2)
                emit_s6mm(i - 2)
            if 0 <= i < NT:
                gg_new = emit_s34a(i)
            else:
                gg_new = None
            if 0 <= i - 2 < NT:
                emit_s6ln(i - 2)
            if 0 <= i - 1 < NT:
                emit_s5mm(i - 1)
            if gg_new is not None:
                emit_s34b(i, gg_new)
    nc.compile()
    return nc


def _prep_host(inputs):
    """Host-side precompute: tables, folded weights, per-core input maps."""
    x = np.asarray(inputs["x"], np.float32)
    done = np.asarray(inputs["done"])
    h0r = np.asarray(inputs["h0_re"], np.float32)
    h0i = np.asarray(inputs["h0_im"], np.float32)
    nu = np.asarray(inputs["nu_log"], np.float64)
    th_log = np.asarray(inputs["theta_log"], np.float64)
    gl = np.asarray(inputs["gamma_log"], np.float64)

    r = np.exp(-np.exp(nu))                     # |lambda|, [N]
    theta = np.exp(th_log)                      # [N]
    gamma = np.exp(gl)

    j = np.arange(CT, dtype=np.float64)
    ang = theta[:, None] * j[None, :]           # [N, CT]
    cosj = np.cos(ang).astype(np.float32)
    sinj = np.sin(ang).astype(np.float32)
    cneg = (-np.cos(ang)).astype(np.float32)
    sneg = (-np.sin(ang)).astype(np.float32)
    rbc = np.repeat(r.astype(np.float32)[:, None], CT, axis=1)
    angL = theta * CT
    ecl = np.cos(angL).astype(np.float32)[:, None]
    esl = np.sin(angL).astype(np.float32)[:, None]

    # q0 = e^{i theta} * h0  per (b, n)
    c1, s1 = np.cos(theta), np.sin(theta)
    q0r = (c1[None, :] * h0r - s1[None, :] * h0i).astype(np.float32)
    q0i = (c1[None, :] * h0i + s1[None, :] * h0r).astype(np.float32)

    brt = np.ascontiguousarray(
        (np.asarray(inputs["B_re"], np.float64) * gamma[:, None]).T
    ).astype(np.float32)
    bit = np.ascontiguousarray(
        (np.asarray(inputs["B_im"], np.float64) * gamma[:, None]).T
    ).astype(np.float32)
    crt = np.ascontiguousarray(np.asarray(inputs["C_re"], np.float32).T)
    cit = np.ascontiguousarray(np.asarray(inputs["C_im"], np.float32).T)
    dt = np.ascontiguousarray(np.asarray(inputs["D_mat"], np.float32).T)
    mt = np.ascontiguousarray(np.asarray(inputs["mlp_w"], np.float32).T)

    g1v = np.asarray(inputs["ln1_g"], np.float32)
    b1v = np.asarray(inputs["ln1_b"], np.float32)
    g2v = np.asarray(inputs["ln2_g"], np.float32)
    b2v = np.asarray(inputs["ln2_b"], np.float32)
    g3v = np.asarray(inputs["ln3_g"], np.float32)
    b3v = np.asarray(inputs["ln3_b"], np.float32)
    mbv = np.asarray(inputs["mlp_b"], np.float32)

    g1 = not (np.all(g1v == 1) and np.all(b1v == 0))
    g2 = not (np.all(g2v == 1) and np.all(b2v == 0))
    g3 = not (np.all(g3v == 1) and np.all(b3v == 0))
    use_bias = bool(np.any(mbv != 0))
    use_mask = bool(np.any(done))
    flags = (g1, g2, g3, use_bias, use_mask)

    shared = dict(cosj=cosj, sinj=sinj, cneg=cneg, sneg=sneg, rbc=rbc,
                  ecl=ecl, esl=esl, nesl=(-esl), BrT=brt, BiT=bit,
                  CrT=crt, CiT=cit, DT=dt, MT=mt)

    def bc(v):
        return np.ascontiguousarray(np.broadcast_to(v[None, :], (128, D))
                                    ).astype(np.float32)
    if g1:
        shared["g1bc"], shared["b1bc"] = bc(g1v), bc(b1v)
    if g2:
        shared["g2bc"], shared["b2bc"] = bc(g2v), bc(b2v)
    if g3:
        shared["g3bc"], shared["b3bc"] = bc(g3v), bc(b3v)
    if use_bias:
        shared["mbbc"] = bc(mbv)

    in_maps = []
    for core in range(NCORES):
        sl = slice(core * BS, (core + 1) * BS)
        m = dict(shared)
        m["x"] = np.ascontiguousarray(x[sl])
        m["q0r"] = np.ascontiguousarray(q0r[sl])
        m["q0i"] = np.ascontiguousarray(q0i[sl])
        if use_mask:
            mask = 1.0 - done[sl].astype(np.float32)       # [BS, T]
            d0 = (rbc[None, :, 0:1] * mask[:, None, :])    # [BS, N, T]
            m["d0tab"] = np.ascontiguousarray(d0.astype(np.float32))
        in_maps.append(m)
    return flags, in_maps


def _get_program(flags):
    if flags not in _PROG_CACHE:
        _PROG_CACHE[flags] = _build_program(flags)
    return _PROG_CACHE[flags]


def run(inputs, trace=False, **kw):
    flags, in_maps = _prep_host(inputs)
    nc = _get_program(flags)
    res = run_bass_kernel_spmd(nc, in_maps, list(range(NCORES)),
                               trace=trace, **kw)
    out = np.concatenate([res.results[i]["out"] for i in range(NCORES)], axis=0)
    return out, res


def kernel(**inputs):
    out, _ = run(inputs, trace=False)
    return out

